# revision 26
# baseline (speedup 1.0000x reference)
"""Masked-MVN (eye covariance) NLL loss on 8 Trainium2 cores.

loss = 0.5 * ( sum(eps^2 * (y != 0)) / (s * B) + D * (log(2*pi) + log(s)) )
with s = softplus(sigma), B = 256, D = 24*4096.

The heavy part (201 MB masked sum-of-squares) runs data-parallel on 8
NeuronCores (32 batches each); the O(1) scalar epilogue runs on host
(the "all-reduce" of the sharding hint). Per core the shard is viewed as
[128 partitions x 24576] and processed in 12 chunks:

  DMA (8 HWDGE queues, 16 SDMA engines, ~423 GB/s/core measured)
    -> DVE scalar_tensor_tensor: e = (y != 0) * eps, in place
    -> ACT activation(Square, accum_out): per-partition sum of squares

eps/y chunks are packed host-side into one input tensor per core, each
chunk a contiguous [128 x (e|y)] DRAM block, so every chunk arrives in a
single DMA and every engine instruction needs at most one sync wait
(this walrus build rejects instructions with more; see _split_waits).
Measured ~77.3 us/core on TRN2 (~59.5 us is pure DMA at engine rate,
~7 us NEFF/runtime startup, ~5 us first-descriptor latency + last-chunk
compute, ~3 us out-DMA + drain).
"""

import sys

for _p in ("/opt/trn_rl_repo",):
    if _p not in sys.path:
        sys.path.insert(0, _p)

import numpy as np

B, Q, N = 256, 24, 4096
NCORES = 8
BSH = B // NCORES            # 32 batches per core
P = 128                      # SBUF partitions
M = BSH * Q * N // P         # 24576 floats per partition per tensor
BLOCKS = [2048] * 12
assert sum(BLOCKS) == M
NCHUNK = len(BLOCKS)         # 12
NBUF = 8                     # io pool depth == queue count (self-staggering pipeline)
TAILSPLIT = 1                # last chunk's compute in col-slices (DVE/ACT pipeline)
NPART = NCHUNK - 1 + TAILSPLIT
D = Q * N                    # 98304 (MVN event dim)

_CACHE = {}


def _build_nc():
    import concourse.bass as bass
    import concourse.mybir as mybir
    import concourse.tile as tile

    nc = bass.Bass()
    # xy is packed so each chunk is one fully CONTIGUOUS DRAM region of
    # P*2*s floats (partition-major): sequential HBM reads per chunk.
    xy = nc.dram_tensor("xy", [1, P * 2 * M], mybir.dt.float32, kind="ExternalInput")
    out = nc.dram_tensor("out", [P, NPART], mybir.dt.float32, kind="ExternalOutput")

    with tile.TileContext(nc) as tc:
        with (
            tc.tile_pool(name="io", bufs=NBUF) as io_pool,
            tc.tile_pool(name="sq", bufs=2) as sq_pool,
            tc.tile_pool(name="acc", bufs=1) as acc_pool,
        ):
            part = acc_pool.tile([P, NPART], mybir.dt.float32)
            off = 0
            col = 0
            for j, s in enumerate(BLOCKS):
                xyt = io_pool.tile([P, 2 * s], mybir.dt.float32, tag="xy")
                src = xy[0, off : off + P * 2 * s].rearrange("(p c) -> p c", p=P)
                nc.sync.dma_start(xyt[:], src)
                off += P * 2 * s

                # Last chunk: sub-slice so DVE (mask-mult) and ACT
                # (square+reduce) pipeline within it — shortens the
                # after-last-DMA dangle.
                nsub = TAILSPLIT if j == NCHUNK - 1 else 1
                w = s // nsub
                for k in range(nsub):
                    e = xyt[:, k * w : (k + 1) * w]
                    yt = xyt[:, s + k * w : s + (k + 1) * w]
                    # e <- (y != 0) * eps  — one DVE pass, in place
                    nc.vector.scalar_tensor_tensor(
                        e,
                        yt,
                        0.0,
                        e,
                        op0=mybir.AluOpType.not_equal,
                        op1=mybir.AluOpType.mult,
                    )
                    # part[:, col] = sum(e^2) — one ACT pass (fused)
                    sq = sq_pool.tile([P, w], mybir.dt.float32, tag="sq")
                    nc.scalar.activation(
                        sq[:],
                        e,
                        mybir.ActivationFunctionType.Square,
                        accum_out=part[:, col : col + 1],
                    )
                    col += 1
            nc.sync.dma_start(out[:], part[:])

    _split_waits(nc, mybir)
    return nc


def _split_waits(nc, mybir):
    """Walrus codegen in this container only accepts ONE sync wait per
    engine/DMA instruction. Hoist extra waits onto InstNoOp instructions
    inserted just before, on the same engine stream (engines execute
    in order, so wait-on-nop then wait-on-inst is equivalent)."""
    f = nc.m.functions[0]
    for blk in f.blocks:
        fixes = []
        for idx, inst in enumerate(blk.instructions):
            si = getattr(inst, "sync_info", None)
            if si is None or not si.on_wait or len(si.on_wait) <= 1:
                continue
            fixes.append((idx, inst))
        if not fixes:
            continue
        result = list(blk.instructions)
        for idx, inst in reversed(fixes):
            waits = list(inst.sync_info.on_wait)
            nops = []
            for w in waits[:-1]:
                bi = nc.engines[inst.engine].nop(hint="wait-hoist")
                nop_inst = bi.ins
                for b2 in f.blocks:
                    if nop_inst in b2.instructions:
                        b2.instructions.remove(nop_inst)
                        break
                else:
                    raise AssertionError("hoist nop not found in any block")
                nop_inst.sync_info = mybir.SyncInfo(on_wait=[w], on_update=[])
                nops.append(nop_inst)
            inst.sync_info = mybir.SyncInfo(
                on_wait=[waits[-1]], on_update=list(inst.sync_info.on_update)
            )
            result[idx:idx] = nops
        blk.instructions = result


def _pack(eps_t, y_t):
    """[NCORES, 1, P*2*M]: per chunk j a contiguous partition-major block
    [p, (e_j[p] | y_j[p])] so the device reads sequential DRAM."""
    e = np.ascontiguousarray(eps_t, dtype=np.float32).reshape(NCORES, P, M)
    y = np.ascontiguousarray(y_t, dtype=np.float32).reshape(NCORES, P, M)
    xy = np.empty((NCORES, P * 2 * M), dtype=np.float32)
    src = 0
    dst = 0
    for s in BLOCKS:
        blk = xy[:, dst : dst + P * 2 * s].reshape(NCORES, P, 2 * s)
        blk[:, :, 0:s] = e[:, :, src : src + s]
        blk[:, :, s : 2 * s] = y[:, :, src : src + s]
        src += s
        dst += P * 2 * s
    return xy.reshape(NCORES, 1, P * 2 * M)


def _execute(in_maps, trace=False):
    from concourse.bass_utils import run_bass_kernel_spmd

    if "nc" not in _CACHE:
        _CACHE["nc"] = _build_nc()
    nc = _CACHE["nc"]
    return run_bass_kernel_spmd(nc, in_maps, core_ids=list(range(NCORES)), trace=trace)


def kernel(eps_t, y_t, sigma):
    xy = _pack(eps_t, y_t)
    in_maps = [{"xy": xy[i]} for i in range(NCORES)]
    res = None
    for attempt in range(3):
        try:
            res = _execute(in_maps)
            break
        except Exception:
            # transient device faults happen on this axon tunnel; retry
            if attempt == 2:
                raise
            import time

            time.sleep(10)
    total = float(sum(np.asarray(r["out"], dtype=np.float64).sum() for r in res.results))

    sig = float(np.asarray(sigma, dtype=np.float64).reshape(-1)[0])
    # softplus(sigma), numerically stable
    s = np.logaddexp(0.0, sig)
    loss = 0.5 * (total / (s * B) + D * (np.log(2.0 * np.pi) + np.log(s)))
    return np.asarray(loss, dtype=np.float32)


# revision 27
# speedup vs baseline: 1.0113x; 1.0113x over previous
"""Masked-MVN (eye covariance) NLL loss on 8 Trainium2 cores.

loss = 0.5 * ( sum(eps^2 * (y != 0)) / (s * B) + D * (log(2*pi) + log(s)) )
with s = softplus(sigma), B = 256, D = 24*4096.

The heavy part (201 MB masked sum-of-squares) runs data-parallel on 8
NeuronCores (32 batches each); the O(1) scalar epilogue runs on host
(the "all-reduce" of the sharding hint). Per core the shard is viewed as
[128 partitions x 24576] and processed in 12 chunks:

  DMA (8 HWDGE queues, 16 SDMA engines, ~423 GB/s/core measured)
    -> DVE scalar_tensor_tensor: e = (y != 0) * eps, in place
    -> ACT activation(Square, accum_out): per-partition sum of squares

eps/y chunks are packed host-side into one input tensor per core, each
chunk a contiguous [128 x (e|y)] DRAM block, so every chunk arrives in a
single DMA and every engine instruction needs at most one sync wait
(this walrus build rejects instructions with more; see _split_waits).
Measured ~77.3 us/core on TRN2 (~59.5 us is pure DMA at engine rate,
~7 us NEFF/runtime startup, ~5 us first-descriptor latency + last-chunk
compute, ~3 us out-DMA + drain).
"""

import sys

for _p in ("/opt/trn_rl_repo",):
    if _p not in sys.path:
        sys.path.insert(0, _p)

import numpy as np

B, Q, N = 256, 24, 4096
NCORES = 8
BSH = B // NCORES            # 32 batches per core
P = 128                      # SBUF partitions
M = BSH * Q * N // P         # 24576 floats per partition per tensor
BLOCKS = [2048] * 12
assert sum(BLOCKS) == M
NCHUNK = len(BLOCKS)         # 12
NBUF = 8                     # io pool depth == queue count (self-staggering pipeline)
TAILSPLIT = 2                # last chunk's compute in col-slices (DVE/ACT pipeline)
NPART = NCHUNK - 1 + TAILSPLIT
D = Q * N                    # 98304 (MVN event dim)

_CACHE = {}


def _build_nc():
    import concourse.bass as bass
    import concourse.mybir as mybir
    import concourse.tile as tile

    nc = bass.Bass()
    # xy is packed so each chunk is one fully CONTIGUOUS DRAM region of
    # P*2*s floats (partition-major): sequential HBM reads per chunk.
    xy = nc.dram_tensor("xy", [1, P * 2 * M], mybir.dt.float32, kind="ExternalInput")
    out = nc.dram_tensor("out", [P, NPART], mybir.dt.float32, kind="ExternalOutput")

    with tile.TileContext(nc) as tc:
        with (
            tc.tile_pool(name="io", bufs=NBUF) as io_pool,
            tc.tile_pool(name="sq", bufs=2) as sq_pool,
            tc.tile_pool(name="acc", bufs=1) as acc_pool,
        ):
            part = acc_pool.tile([P, NPART], mybir.dt.float32)
            off = 0
            col = 0
            for j, s in enumerate(BLOCKS):
                xyt = io_pool.tile([P, 2 * s], mybir.dt.float32, tag="xy")
                src = xy[0, off : off + P * 2 * s].rearrange("(p c) -> p c", p=P)
                nc.sync.dma_start(xyt[:], src)
                off += P * 2 * s

                # Last chunk: sub-slice so DVE (mask-mult) and ACT
                # (square+reduce) pipeline within it — shortens the
                # after-last-DMA dangle.
                nsub = TAILSPLIT if j == NCHUNK - 1 else 1
                w = s // nsub
                for k in range(nsub):
                    e = xyt[:, k * w : (k + 1) * w]
                    yt = xyt[:, s + k * w : s + (k + 1) * w]
                    # e <- (y != 0) * eps  — one DVE pass, in place
                    nc.vector.scalar_tensor_tensor(
                        e,
                        yt,
                        0.0,
                        e,
                        op0=mybir.AluOpType.not_equal,
                        op1=mybir.AluOpType.mult,
                    )
                    # part[:, col] = sum(e^2) — one ACT pass (fused)
                    sq = sq_pool.tile([P, w], mybir.dt.float32, tag="sq")
                    nc.scalar.activation(
                        sq[:],
                        e,
                        mybir.ActivationFunctionType.Square,
                        accum_out=part[:, col : col + 1],
                    )
                    col += 1
            nc.sync.dma_start(out[:], part[:])

    _split_waits(nc, mybir)
    return nc


def _split_waits(nc, mybir):
    """Walrus codegen in this container only accepts ONE sync wait per
    engine/DMA instruction. Hoist extra waits onto InstNoOp instructions
    inserted just before, on the same engine stream (engines execute
    in order, so wait-on-nop then wait-on-inst is equivalent)."""
    f = nc.m.functions[0]
    for blk in f.blocks:
        fixes = []
        for idx, inst in enumerate(blk.instructions):
            si = getattr(inst, "sync_info", None)
            if si is None or not si.on_wait or len(si.on_wait) <= 1:
                continue
            fixes.append((idx, inst))
        if not fixes:
            continue
        result = list(blk.instructions)
        for idx, inst in reversed(fixes):
            waits = list(inst.sync_info.on_wait)
            nops = []
            for w in waits[:-1]:
                bi = nc.engines[inst.engine].nop(hint="wait-hoist")
                nop_inst = bi.ins
                for b2 in f.blocks:
                    if nop_inst in b2.instructions:
                        b2.instructions.remove(nop_inst)
                        break
                else:
                    raise AssertionError("hoist nop not found in any block")
                nop_inst.sync_info = mybir.SyncInfo(on_wait=[w], on_update=[])
                nops.append(nop_inst)
            inst.sync_info = mybir.SyncInfo(
                on_wait=[waits[-1]], on_update=list(inst.sync_info.on_update)
            )
            result[idx:idx] = nops
        blk.instructions = result


def _pack(eps_t, y_t):
    """[NCORES, 1, P*2*M]: per chunk j a contiguous partition-major block
    [p, (e_j[p] | y_j[p])] so the device reads sequential DRAM."""
    e = np.ascontiguousarray(eps_t, dtype=np.float32).reshape(NCORES, P, M)
    y = np.ascontiguousarray(y_t, dtype=np.float32).reshape(NCORES, P, M)
    xy = np.empty((NCORES, P * 2 * M), dtype=np.float32)
    src = 0
    dst = 0
    for s in BLOCKS:
        blk = xy[:, dst : dst + P * 2 * s].reshape(NCORES, P, 2 * s)
        blk[:, :, 0:s] = e[:, :, src : src + s]
        blk[:, :, s : 2 * s] = y[:, :, src : src + s]
        src += s
        dst += P * 2 * s
    return xy.reshape(NCORES, 1, P * 2 * M)


def _execute(in_maps, trace=False):
    from concourse.bass_utils import run_bass_kernel_spmd

    if "nc" not in _CACHE:
        _CACHE["nc"] = _build_nc()
    nc = _CACHE["nc"]
    return run_bass_kernel_spmd(nc, in_maps, core_ids=list(range(NCORES)), trace=trace)


def kernel(eps_t, y_t, sigma):
    xy = _pack(eps_t, y_t)
    in_maps = [{"xy": xy[i]} for i in range(NCORES)]
    res = None
    for attempt in range(3):
        try:
            res = _execute(in_maps)
            break
        except Exception:
            # transient device faults happen on this axon tunnel; retry
            if attempt == 2:
                raise
            import time

            time.sleep(10)
    total = float(sum(np.asarray(r["out"], dtype=np.float64).sum() for r in res.results))

    sig = float(np.asarray(sigma, dtype=np.float64).reshape(-1)[0])
    # softplus(sigma), numerically stable
    s = np.logaddexp(0.0, sig)
    loss = 0.5 * (total / (s * B) + D * (np.log(2.0 * np.pi) + np.log(s)))
    return np.asarray(loss, dtype=np.float32)
